# revision 1
# baseline (speedup 1.0000x reference)
"""Trainium2 Bass kernel for nn_AdaptiveAutoCorrelation (8-core data-parallel).

Per-core (one batch element b):
  1. LayerNorm(q), LayerNorm(k) over E=64 segments in mega-tile form
     ([128, 12*512] per tensor); stats on DVE, square/sqrt on ACT.
  2. Avg-pool to scales 2,4 via PE matmuls (P2a/P2b packing matrices).
  3. rFFT of q,k per scale as float32r matmuls against host-built DFT matrices;
     PSUM drained to SBUF by ACT immediately (4-deep rotation keeps PE dense).
  4. Spectral whitening qf*conj(kf)/|kf| + (h,e)-reduction via stt accum -> S[f].
  5. mean_corr = S @ M (irfft+interp+scale-weights+mean folded into M), matmuls
     emitted one pair behind the DFT stream so PE never waits on DVE.
  6. top-7 via DVE max/max_index, softmax; 7 indirect gathers from a host-built
     sliding-window bf16 buffer (vvwide[i] = v[i], v[i+128], ..., v[i+1408]),
     one [128, 6144] gather per delay + weighted MAC on DVE.

The (H,E)-mean/clip swap is exact for this model: |corr| <= ~3.7 << 10.
"""
import math

import numpy as np

L = 1536
H, E = 8, 64
R = H * E  # 512
B = 8
NT = L // 128  # 12 l-tiles
SCALES = [1, 2, 4]
KT = [12, 6, 3]  # contraction tiles per scale (pooled-first)
FBINS = [L // s // 2 + 1 for s in SCALES]  # [769, 385, 193]
FT = [(f + 127) // 128 for f in FBINS]  # f-tiles per re/im block: [7, 4, 2]
NFT = 2 * sum(FT)  # 26 total f-tiles
TOPK = int(math.log(L))  # 7
LN_EPS = 1e-5
GPK = 4  # tiles packed per gather row (4KB bf16 rows - verified on HW)
NGRP = NT // GPK  # 3 gathers per delay
NW = 2 * L - 128 * (GPK - 1)  # 2688 rows in the sliding-window gather buffer

# global ftile index bases (for S / M layout): per scale, re tiles then im tiles
_FT_BASE = []
_acc = 0
for _s in range(len(SCALES)):
    _FT_BASE.append((_acc, _acc + FT[_s]))
    _acc += 2 * FT[_s]

# flat D-tile index: for si, local_ft in [0, 2*FT[si]), kt in [0, KT[si])
_D_IDX = {}
_n = 0
for _si in range(len(SCALES)):
    for _lf in range(2 * FT[_si]):
        for _kt in range(KT[_si]):
            _D_IDX[(_si, _lf, _kt)] = _n
            _n += 1
ND_TILES = _n  # 228

_CACHE = {}


def _build_constants(scale_weights, frequency_filter):
    """D tiles [ND_TILES,128,128], M tiles [NFT,3,128,512], pool mats [2,128,128]."""
    f_sig = 1.0 / (1.0 + np.exp(-np.float64(frequency_filter[0])))
    sw = np.asarray(scale_weights[: len(SCALES)], np.float64)
    w = np.exp(sw - sw.max())
    w = w / w.sum()

    D_t = np.zeros((ND_TILES, 128, 128), np.float32)
    M = np.zeros((NFT * 128, L), np.float64)
    for si, s in enumerate(SCALES):
        Ls = L // s
        F = FBINS[si]
        nf = FT[si]
        t = np.arange(Ls)[:, None]
        f = np.arange(F)[None, :]
        ang = 2.0 * np.pi * t * f / Ls
        Dre = np.zeros((Ls, nf * 128))
        Dim = np.zeros((Ls, nf * 128))
        Dre[:, :F] = np.cos(ang)
        Dim[:, :F] = -np.sin(ang)
        for lf in range(2 * nf):
            blk = Dre if lf < nf else Dim
            j = lf % nf
            for kt in range(KT[si]):
                D_t[_D_IDX[(si, lf, kt)]] = blk[
                    kt * 128 : (kt + 1) * 128, j * 128 : (j + 1) * 128
                ].astype(np.float32)

        reb, imb = _FT_BASE[si]
        tt = np.arange(Ls)[None, :]
        cf = np.where((f.T == 0) | (f.T == F - 1), 1.0, 2.0)
        ang2 = 2.0 * np.pi * f.T * tt / Ls
        Mre = cf * np.cos(ang2) / Ls  # [F, Ls]
        Mim = -cf * np.sin(ang2) / Ls
        if Ls != L:
            P = np.zeros((Ls, L))
            co = np.clip((np.arange(L) + 0.5) * (Ls / L) - 0.5, 0, Ls - 1)
            lo = np.floor(co).astype(int)
            hi = np.minimum(lo + 1, Ls - 1)
            fr = co - lo
            P[lo, np.arange(L)] += 1 - fr
            P[hi, np.arange(L)] += fr
            Mre = Mre @ P
            Mim = Mim @ P
        scale = w[si] * f_sig / R
        M[reb * 128 : reb * 128 + F] = Mre * scale
        M[imb * 128 : imb * 128 + F] = Mim * scale

    M_t = (
        M.reshape(NFT, 128, 3, 512).transpose(0, 2, 1, 3).astype(np.float32).copy()
    )
    # pool-by-2 packing matrices: P2a -> out cols [0,64), P2b -> [64,128)
    P2 = np.zeros((2, 128, 128), np.float32)
    for t_ in range(128):
        P2[0, t_, t_ // 2] = 0.5
        P2[1, t_, 64 + t_ // 2] = 0.5
    return D_t, M_t, P2


def _build_graph():
    import concourse.bacc as bacc
    import concourse.bass as bass
    import concourse.mybir as mybir
    import concourse.tile as tile

    AF = mybir.ActivationFunctionType
    OP = mybir.AluOpType
    f32 = mybir.dt.float32
    f32r = mybir.dt.float32r
    bf16 = mybir.dt.bfloat16
    u32 = mybir.dt.uint32

    nc = bacc.Bacc("TRN2", debug=False)
    q_d = nc.dram_tensor("q", [NT, 128, R], f32, kind="ExternalInput")
    k_d = nc.dram_tensor("k", [NT, 128, R], f32, kind="ExternalInput")
    vw_d = nc.dram_tensor("vw", [NW, GPK * R], bf16, kind="ExternalInput")
    d_d = nc.dram_tensor("dmat", [ND_TILES, 128, 128], f32r, kind="ExternalInput")
    m_d = nc.dram_tensor("mmat", [NFT, 3, 128, 512], f32r, kind="ExternalInput")
    p_d = nc.dram_tensor("pmat", [2, 128, 128], f32r, kind="ExternalInput")
    o_d = nc.dram_tensor("out", [NT, 128, R], f32, kind="ExternalOutput")

    def r32(ap):
        return ap.bitcast(f32r)

    with tile.TileContext(nc) as tc:
        with (
            tc.tile_pool(name="qk", bufs=1) as qkpool,
            tc.tile_pool(name="small", bufs=1) as spool,
        ):
            eps_ln = spool.tile([128, 1], f32, tag="eps_ln")
            nc.vector.memset(eps_ln[:], LN_EPS)
            eps_mag = spool.tile([128, 1], f32, tag="eps_mag")
            nc.vector.memset(eps_mag[:], 1e-30)
            p2a = spool.tile([128, 128], f32r, tag="p2a")
            p2b = spool.tile([128, 128], f32r, tag="p2b")
            nc.sync.dma_start(p2a[:], p_d.ap()[0])
            nc.sync.dma_start(p2b[:], p_d.ap()[1])

            # ---- load + layernorm (mega-tile per tensor) ----
            # xn[(name, si)] = mega tile [128, nkt, 512]; rhs slices per kt
            xn = {}
            with tc.tile_pool(name="lnwork", bufs=2) as wpool:
                for name, src in (("q", q_d), ("k", k_d)):
                    raw = wpool.tile([128, NT, R], f32, tag="raw")
                    nc.sync.dma_start(
                        raw[:], src.ap().rearrange("t p r -> p t r")
                    )
                    x4 = raw[:].rearrange("p t (h e) -> p t h e", e=E)
                    sq = wpool.tile([128, NT, R], f32, tag="sq")
                    nc.scalar.activation(sq[:], raw[:], AF.Square)
                    stat = wpool.tile([128, 96], f32, tag="stat")
                    nc.vector.tensor_reduce(
                        stat[:], x4, mybir.AxisListType.X, OP.add
                    )
                    msq = wpool.tile([128, 96], f32, tag="msq")
                    nc.vector.tensor_reduce(
                        msq[:], sq[:].rearrange("p t (h e) -> p t h e", e=E),
                        mybir.AxisListType.X, OP.add,
                    )
                    mean = wpool.tile([128, 96], f32, tag="mean")
                    nc.vector.tensor_scalar_mul(mean[:], stat[:], 1.0 / E)
                    m2 = wpool.tile([128, 96], f32, tag="m2")
                    nc.vector.tensor_mul(m2[:], mean[:], mean[:])
                    var = wpool.tile([128, 96], f32, tag="var")
                    nc.vector.scalar_tensor_tensor(
                        var[:], msq[:], 1.0 / E, m2[:], op0=OP.mult, op1=OP.subtract
                    )
                    std = wpool.tile([128, 96], f32, tag="std")
                    nc.scalar.activation(std[:], var[:], AF.Sqrt, bias=eps_ln[:])
                    rstd = wpool.tile([128, 96], f32, tag="rstd")
                    nc.vector.reciprocal(rstd[:], std[:])
                    mega = qkpool.tile(
                        [128, NT, R], f32, tag=f"{name}mega", name=f"{name}mega"
                    )
                    mg4 = mega[:].rearrange("p t (h e) -> p t h e", e=E)
                    mean4 = mean[:].rearrange("p (t h o) -> p t h o", t=NT, o=1)
                    rstd4 = rstd[:].rearrange("p (t h o) -> p t h o", t=NT, o=1)
                    x4b, mean_b = bass.broadcast_tensor_aps(x4, mean4)
                    nc.vector.tensor_tensor(r32(mg4), x4b, mean_b, OP.subtract)
                    _, rstd_b = bass.broadcast_tensor_aps(mg4, rstd4)
                    nc.vector.tensor_tensor(r32(mg4), mg4, rstd_b, OP.mult)
                    xn[(name, 0)] = mega

            # ---- avg-pool to scales 2 and 4 via PE ----
            with tc.tile_pool(name="poolps", bufs=4, space="PSUM") as ppool:
                for name in ("q", "k"):
                    for si, nkt in ((1, 6), (2, 3)):
                        mega = qkpool.tile(
                            [128, nkt, R], f32r, tag=f"{name}p{si}",
                            name=f"{name}p{si}",
                        )
                        srcm = xn[(name, si - 1)]
                        for j in range(nkt):
                            ps = ppool.tile([128, R], f32, tag="ps", name="ps")
                            nc.tensor.matmul(
                                ps[:], p2a[:], r32(srcm[:, 2 * j, :]),
                                start=True, stop=False,
                            )
                            nc.tensor.matmul(
                                ps[:], p2b[:], r32(srcm[:, 2 * j + 1, :]),
                                start=False, stop=True,
                            )
                            nc.scalar.activation(mega[:, j, :], ps[:], AF.Copy)
                        xn[(name, si)] = mega

            # ---- DFT + spectral + pipelined irfft (mean_corr) ----
            S_big = spool.tile([128, 32], f32, tag="sbig")
            S_r32 = spool.tile([128, 32], f32r, tag="sr32")
            with (
                tc.tile_pool(name="psum", bufs=4, space="PSUM") as pp,
                tc.tile_pool(name="mcpsum", bufs=1, space="PSUM") as mcp,
                tc.tile_pool(name="dstream", bufs=8) as dpool,
                tc.tile_pool(name="spec", bufs=2) as scp,
            ):
                mc_ps = [
                    mcp.tile([1, 512], f32, tag=f"mc{nt}", name=f"mc{nt}")
                    for nt in range(3)
                ]
                pair_list = []
                for si in range(len(SCALES)):
                    reb, imb = _FT_BASE[si]
                    for j in range(FT[si]):
                        pair_list.append((si, j, reb + j, imb + j))
                n_pairs = len(pair_list)

                def emit_mc(pi2, first_mm):
                    si2, j2, ftr2, fti2 = pair_list[pi2]
                    for ft in (ftr2, fti2):
                        for nt in range(3):
                            mtile = dpool.tile([128, 512], f32r, tag="mtile")
                            nc.sync.dma_start(mtile[:], m_d.ap()[ft, nt])
                            nc.tensor.matmul(
                                mc_ps[nt][:], S_r32[:, ft : ft + 1], mtile[:],
                                start=first_mm,
                                stop=(pi2 == n_pairs - 1 and ft == fti2 and nt == 2),
                                skip_group_check=True,
                            )
                        first_mm = False
                    return first_mm

                first_mm = True
                for pi, (si, j, ftr, fti) in enumerate(pair_list):
                    nkt = KT[si]
                    qx = xn[("q", si)]
                    kx = xn[("k", si)]
                    psl = {}
                    for nm, xm, lf in (
                        ("qre", qx, j), ("kre", kx, j),
                        ("qim", qx, FT[si] + j), ("kim", kx, FT[si] + j),
                    ):
                        ps = pp.tile([128, 512], f32, tag="dftps", name=f"ps{nm}")
                        for kt in range(nkt):
                            dt_ = dpool.tile(
                                [128, 128], f32r,
                                tag="dre" if lf == j else "dim",
                            )
                            nc.sync.dma_start(dt_[:], d_d.ap()[_D_IDX[(si, lf, kt)]])
                            nc.tensor.matmul(
                                ps[:], dt_[:], r32(xm[:, kt, :]),
                                start=(kt == 0), stop=(kt == nkt - 1),
                            )
                        sb = scp.tile([128, 512], f32, tag=f"{nm}S")
                        nc.scalar.activation(sb[:], ps[:], AF.Copy)
                        psl[nm] = sb
                    # mc matmuls for the previous pair (PE stays dense)
                    if pi > 0:
                        first_mm = emit_mc(pi - 1, first_mm)
                    qreS, qimS = psl["qre"], psl["qim"]
                    kreS, kimS = psl["kre"], psl["kim"]
                    t1 = scp.tile([128, 512], f32, tag="t1")
                    t2 = scp.tile([128, 512], f32, tag="t2")
                    nc.scalar.activation(t1[:], kreS[:], AF.Square)
                    nc.scalar.activation(t2[:], kimS[:], AF.Square)
                    nc.vector.tensor_add(t1[:], t1[:], t2[:])
                    nc.scalar.activation(t1[:], t1[:], AF.Sqrt, bias=eps_mag[:, 0:1])
                    rs = scp.tile([128, 512], f32, tag="rs")
                    nc.vector.reciprocal(rs[:], t1[:])
                    khr = scp.tile([128, 512], f32, tag="khr")
                    khi = scp.tile([128, 512], f32, tag="khi")
                    nc.vector.tensor_mul(khr[:], kreS[:], rs[:])
                    nc.vector.tensor_mul(khi[:], kimS[:], rs[:])
                    scr = scp.tile([128, 512], f32, tag="scr")
                    scr2 = scp.tile([128, 512], f32, tag="scr2")
                    a1 = scp.tile([128, 1], f32, tag="a1")
                    a2 = scp.tile([128, 1], f32, tag="a2")
                    a3 = scp.tile([128, 1], f32, tag="a3")
                    a4 = scp.tile([128, 1], f32, tag="a4")
                    nc.vector.scalar_tensor_tensor(
                        scr[:], qreS[:], 0.0, khr[:], op0=OP.bypass, op1=OP.mult,
                        accum_out=a1[:],
                    )
                    nc.vector.scalar_tensor_tensor(
                        scr2[:], qimS[:], 0.0, khi[:], op0=OP.bypass, op1=OP.mult,
                        accum_out=a2[:],
                    )
                    nc.vector.tensor_add(S_big[:, ftr : ftr + 1], a1[:], a2[:])
                    nc.vector.scalar_tensor_tensor(
                        scr[:], qimS[:], 0.0, khr[:], op0=OP.bypass, op1=OP.mult,
                        accum_out=a3[:],
                    )
                    nc.vector.scalar_tensor_tensor(
                        scr2[:], qreS[:], 0.0, khi[:], op0=OP.bypass, op1=OP.mult,
                        accum_out=a4[:],
                    )
                    nc.vector.tensor_sub(S_big[:, fti : fti + 1], a3[:], a4[:])
                    nc.vector.tensor_copy(
                        S_r32[:, ftr : ftr + 1], S_big[:, ftr : ftr + 1]
                    )
                    nc.vector.tensor_copy(
                        S_r32[:, fti : fti + 1], S_big[:, fti : fti + 1]
                    )
                first_mm = emit_mc(n_pairs - 1, first_mm)

                mc_row = spool.tile([1, L], f32, tag="mcrow")
                for nt in range(3):
                    nc.vector.tensor_copy(
                        mc_row[:, nt * 512 : (nt + 1) * 512], mc_ps[nt][:]
                    )

            # ---- top-7 + softmax ----
            mc8 = spool.tile([1, 8], f32, tag="mc8")
            mcidx = spool.tile([1, 8], u32, tag="mcidx")
            nc.vector.max(mc8[:], mc_row[:])
            nc.vector.max_index(mcidx[:], mc8[:], mc_row[:])
            mc8c = spool.tile([1, 8], f32, tag="mc8c")
            nc.vector.tensor_copy(mc8c[:], mc8[:])
            mcidxc = spool.tile([1, 8], u32, tag="mcidxc")
            nc.vector.tensor_copy(mcidxc[:], mcidx[:])
            negmax = spool.tile([1, 1], f32, tag="negmax")
            nc.vector.tensor_scalar_mul(negmax[:], mc8c[:, 0:1], -1.0)
            e7 = spool.tile([1, TOPK], f32, tag="e7")
            nc.scalar.activation(e7[:], mc8c[:, 0:TOPK], AF.Exp, bias=negmax[:])
            ssum = spool.tile([1, 1], f32, tag="ssum")
            nc.vector.tensor_reduce(ssum[:], e7[:], mybir.AxisListType.X, OP.add)
            rsum = spool.tile([1, 1], f32, tag="rsum")
            nc.vector.reciprocal(rsum[:], ssum[:])
            nw = spool.tile([1, TOPK], f32, tag="nw")
            nc.vector.tensor_scalar_mul(nw[:], e7[:], rsum[:, 0:1])
            nw128 = spool.tile([128, TOPK], f32, tag="nw128")
            nc.gpsimd.partition_broadcast(nw128[:], nw[:])
            d128a = spool.tile([128, TOPK], u32, tag="d128a")
            nc.gpsimd.partition_broadcast(d128a[:], mcidxc[:, 0:TOPK])
            iotas = []
            for g in range(NGRP):
                it = spool.tile([128, 1], u32, tag=f"iota{g}", name=f"iota{g}")
                nc.gpsimd.iota(
                    it[:], pattern=[[0, 1]], base=128 * GPK * g, channel_multiplier=1
                )
                iotas.append(it)

            # ---- gather (3 packed indirect gathers per delay) + MAC ----
            with tc.tile_pool(name="gather", bufs=4) as gpool:
                acc = gpool.tile([128, NT, R], f32, tag="acc", bufs=1)
                for kk in range(TOPK):
                    for g in range(NGRP):
                        idx = gpool.tile([128, 1], u32, tag="idx")
                        nc.vector.tensor_tensor(
                            idx[:], iotas[g][:], d128a[:, kk : kk + 1], OP.add
                        )
                        slot = gpool.tile(
                            [128, GPK * R], bf16, tag="slot", bufs=4
                        )
                        nc.gpsimd.indirect_dma_start(
                            out=slot[:],
                            out_offset=None,
                            in_=vw_d.ap(),
                            in_offset=bass.IndirectOffsetOnAxis(ap=idx[:, 0:1], axis=0),
                        )
                        av = acc[:].rearrange("p t r -> p (t r)")[
                            :, GPK * R * g : GPK * R * (g + 1)
                        ]
                        if kk == 0:
                            nc.vector.tensor_scalar_mul(av, slot[:], nw128[:, 0:1])
                        else:
                            nc.vector.scalar_tensor_tensor(
                                av, slot[:], nw128[:, kk : kk + 1], av,
                                op0=OP.mult, op1=OP.add,
                            )
                for kt in range(NT):
                    nc.sync.dma_start(o_d.ap()[kt], acc[:, kt, :])

    nc.compile()
    return nc


def _get_graph():
    if "nc" not in _CACHE:
        _CACHE["nc"] = _build_graph()
    return _CACHE["nc"]


def _make_in_maps(queries, keys, values, scale_weights, frequency_filter):
    import ml_dtypes

    D_t, M_t, P2 = _build_constants(
        np.asarray(scale_weights, np.float64), np.asarray(frequency_filter, np.float64)
    )
    q = np.ascontiguousarray(np.asarray(queries, np.float32).reshape(B, NT, 128, R))
    k = np.ascontiguousarray(np.asarray(keys, np.float32).reshape(B, NT, 128, R))
    v = np.asarray(values, np.float32).reshape(B, L, R)
    vv = np.concatenate([v, v], axis=1).astype(ml_dtypes.bfloat16)  # [B, 2L, R]
    # sliding-window buffer: vw[b, i, c, :] = vv[b, i + 128*c, :], c < GPK
    st = vv.strides
    vw = np.lib.stride_tricks.as_strided(
        vv, shape=(B, NW, GPK, R), strides=(st[0], st[1], 128 * st[1], st[2])
    )
    in_maps = []
    for b in range(B):
        in_maps.append(
            {
                "q": q[b],
                "k": k[b],
                "vw": np.ascontiguousarray(vw[b]).reshape(NW, GPK * R),
                "dmat": D_t,
                "mmat": M_t,
                "pmat": P2,
            }
        )
    return in_maps


def kernel(queries, keys, values, scale_weights, frequency_filter, attn_mask=None):
    from concourse.bass_utils import run_bass_kernel_spmd

    nc = _get_graph()
    in_maps = _make_in_maps(queries, keys, values, scale_weights, frequency_filter)
    res = run_bass_kernel_spmd(nc, in_maps, core_ids=list(range(B)))
    out = np.stack([res.results[b]["out"].reshape(L, H, E) for b in range(B)])
    return out.astype(np.float32)

